# revision 11
# baseline (speedup 1.0000x reference)
"""Trainium2 Bass kernel for a class-weighted focal loss (CLASSNetLoss).

Reference math (per element, p = clip(x, 1e-5, 0.99999), w_c = c+1):
    pos = -(SS - w) * log(p) * (1-p)^2      if t > 0
    neg = -w       * log(1-p) * p^2         if t == 0
    out = 10 * mean(where(t>0, pos, neg) / SS),  SS = 210

Key reformulation (t in {0,1}):
    r = t ? p : (1-p) = |1 - x - t|          (one select instead of two logs)
    E = clip(log r) * (1-r)^2                (shared by both branches)
    raw = -(SS*t + w*(1-2t)) * E
    sum(raw) = -SS*sum(t*E) - sum_c w_c * colsum((1-2t)*E)_c

So each core only needs two per-class column sums: colsum(E) and
colsum((1-2t)/2 * E); the host applies the class weights.

Per-core layout: batch shard [65536, 20] viewed as [128 partitions x 10240],
where free index f has class (f mod 20).  Column sums are computed on the
tensor engine with a ones[128,1] stationary operand, accumulating N=320
column blocks (320 % 20 == 0) into a single PSUM [1,320] accumulator, so
PSUM column j holds class (j mod 20).

Engine budget per core (10240 elem/partition): DMA ~30us (HBM-bound, the
roofline), ACT 3 passes (Abs/Ln/Square, all in the `natural_log` table set),
DVE 3 passes (bf16), PE 64 small matmuls.
"""

from contextlib import ExitStack

import numpy as np

import concourse.bass as bass
import concourse.bacc as bacc
import concourse.tile as tile
from concourse import mybir
from concourse.bass_utils import run_bass_kernel_spmd

B, C = 524288, 20
NCORES = 8
BS = B // NCORES            # 65536 batch rows per core
P = 128                     # SBUF partitions
FD = BS * C // P            # 10240 free elements per partition
CHUNK = 2560                # free elems per pipeline chunk (multiple of 20)
NCH = FD // CHUNK           # 4 chunks
NMM = 320                   # matmul free size (multiple of 20, <=512)
SS = 210.0
LOG_LO = float(np.log(np.float32(1e-5)))    # -11.512925
W = np.arange(1, C + 1, dtype=np.float64)   # class weights

F32 = mybir.dt.float32
BF16 = mybir.dt.bfloat16
I32 = mybir.dt.int32
Alu = mybir.AluOpType
Act = mybir.ActivationFunctionType


def build_bass(
    stt_engine: str = "gpsimd", reps: int = 1, loop_n: int = 0
) -> bass.Bass:
    """Per-core SPMD program.

    `reps` statically unrolls the body; `loop_n` > 0 additionally wraps the
    body in a dynamic For_i loop (used only for timing amplification).
    """
    nc = bacc.Bacc(None, debug=False)
    x = nc.dram_tensor("output", [BS, C], F32, kind="ExternalInput")
    t = nc.dram_tensor("target", [BS, C], I32, kind="ExternalInput")
    out = nc.dram_tensor("partials", [1, 2 * NMM], F32, kind="ExternalOutput")

    xv = x[:].rearrange("(p f) c -> p (f c)", p=P)   # [128, 10240] f32
    tv = t[:].rearrange("(p f) c -> p (f c)", p=P)   # [128, 10240] i32

    with ExitStack() as ctx:
        tc = ctx.enter_context(tile.TileContext(nc))
        singles = ctx.enter_context(tc.tile_pool(name="singles", bufs=1))
        apool = ctx.enter_context(tc.tile_pool(name="a", bufs=3))
        tpool = ctx.enter_context(tc.tile_pool(name="t", bufs=3))
        upool = ctx.enter_context(tc.tile_pool(name="u", bufs=2))
        bpool = ctx.enter_context(tc.tile_pool(name="b", bufs=2))
        opool = ctx.enter_context(tc.tile_pool(name="o", bufs=2))
        psum = ctx.enter_context(tc.tile_pool(name="ps", bufs=2, space="PSUM"))

        ones = singles.tile([P, 1], BF16)
        nc.vector.memset(ones, 1.0)

        def body():
          for rep in range(reps):
            ps_e = psum.tile([1, NMM], F32, tag="ps_e")
            ps_m = psum.tile([1, NMM], F32, tag="ps_m")

            for ci in range(NCH):
                sl = slice(ci * CHUNK, (ci + 1) * CHUNK)
                xt = apool.tile([P, CHUNK], F32, tag="xt")
                ti = tpool.tile([P, CHUNK], I32, tag="ti")
                nc.sync.dma_start(out=xt, in_=xv[:, sl])
                nc.sync.dma_start(out=ti, in_=tv[:, sl])

                # u = x + t   (so |1-u| = t ? x : 1-x)
                u = upool.tile([P, CHUNK], F32, tag="u")
                if stt_engine == "gpsimd":
                    nc.gpsimd.tensor_add(u, xt, ti)
                else:
                    nc.vector.scalar_tensor_tensor(
                        out=u, in0=xt, scalar=0.0, in1=ti,
                        op0=Alu.add, op1=Alu.add,
                    )

                # r = |1 - u|  (keep f32: feeds both transcendentals)
                r = upool.tile([P, CHUNK], F32, tag="r")
                nc.scalar.activation(r, u, Act.Abs, bias=1.0, scale=-1.0)

                # L0 = ln(r)   (bf16; -inf at r==0 is clamped in the stt below)
                l0 = bpool.tile([P, CHUNK], BF16, tag="l0")
                nc.scalar.activation(l0, r, Act.Ln)

                # s = (1 - r)^2
                s = bpool.tile([P, CHUNK], BF16, tag="s")
                nc.scalar.activation(s, r, Act.Square, bias=1.0, scale=-1.0)

                # g = 0.5 - t = (1-2t)/2     (DVE tensor_scalar on int input)
                g = bpool.tile([P, CHUNK], BF16, tag="g")
                nc.vector.tensor_scalar(
                    out=g, in0=ti, scalar1=-1.0, scalar2=0.5,
                    op0=Alu.mult, op1=Alu.add,
                )

                # E = max(L0, log(1e-5)) * s
                e = bpool.tile([P, CHUNK], BF16, tag="e")
                nc.vector.scalar_tensor_tensor(
                    out=e, in0=l0, scalar=LOG_LO, in1=s,
                    op0=Alu.max, op1=Alu.mult,
                )

                # M2 = E * g = (1-2t)/2 * E
                m2 = bpool.tile([P, CHUNK], BF16, tag="m2")
                nc.vector.tensor_mul(m2, e, g)

                # Column-sum both onto PSUM via ones[128,1].T @ rhs[128,320].
                for j in range(CHUNK // NMM):
                    js = slice(j * NMM, (j + 1) * NMM)
                    first = ci == 0 and j == 0
                    last = ci == NCH - 1 and j == CHUNK // NMM - 1
                    nc.tensor.matmul(
                        ps_e[0:1, :], ones, e[:, js], start=first, stop=last
                    )
                    nc.tensor.matmul(
                        ps_m[0:1, :], ones, m2[:, js], start=first, stop=last
                    )

            res = opool.tile([1, 2 * NMM], F32, tag="res")
            nc.vector.tensor_copy(res[0:1, 0:NMM], ps_e[0:1, :])
            nc.vector.tensor_copy(res[0:1, NMM : 2 * NMM], ps_m[0:1, :])
            nc.sync.dma_start(out=out[:], in_=res)

        if loop_n > 0:
            with tc.For_i(0, loop_n, 1):
                body()
        else:
            body()

    nc.finalize()
    return nc


_NC_CACHE: dict = {}


def _get_nc(**kw) -> bass.Bass:
    key = tuple(sorted(kw.items()))
    if key not in _NC_CACHE:
        _NC_CACHE[key] = build_bass(**kw)
    return _NC_CACHE[key]


def combine_partials(partials: list[np.ndarray]) -> np.float32:
    """Host-side reduction of the per-core [1, 640] partial sums."""
    cs_e = np.zeros(C, dtype=np.float64)
    cs_m2 = np.zeros(C, dtype=np.float64)
    cols = np.arange(NMM) % C
    for p in partials:
        p = np.asarray(p, dtype=np.float64).reshape(2 * NMM)
        np.add.at(cs_e, cols, p[:NMM])
        np.add.at(cs_m2, cols, p[NMM:])
    cs_me = 2.0 * cs_m2                 # colsum((1-2t) * E)
    cs_te = (cs_e - cs_me) / 2.0        # colsum(t * E)
    total = (-SS * cs_te - W * cs_me).sum()
    return np.float32(10.0 * total / (SS * B * C))


def kernel(output: np.ndarray, target: np.ndarray) -> np.ndarray:
    output = np.ascontiguousarray(np.asarray(output, dtype=np.float32))
    target = np.ascontiguousarray(np.asarray(target, dtype=np.int32))
    assert output.shape == (B, C) and target.shape == (B, C)

    nc = _get_nc()
    xs = output.reshape(NCORES, BS, C)
    ts = target.reshape(NCORES, BS, C)
    in_maps = [{"output": xs[i], "target": ts[i]} for i in range(NCORES)]
    res = run_bass_kernel_spmd(nc, in_maps, core_ids=list(range(NCORES)))
    return np.asarray(
        combine_partials([res.results[i]["partials"] for i in range(NCORES)])
    )


# revision 29
# speedup vs baseline: 1.2390x; 1.2390x over previous
"""Trainium2 Bass kernel for a class-weighted focal loss (CLASSNetLoss).

Reference math (per element, p = clip(x, 1e-5, 0.99999), w_c = c+1):
    pos = -(SS - w) * log(p) * (1-p)^2      if t > 0
    neg = -w       * log(1-p) * p^2         if t == 0
    out = 10 * mean(where(t>0, pos, neg) / SS),  SS = 210

Key reformulation (t in {0,1}):
    r   = t ? p : (1-p) = |x + t - 1|        (one select => ONE log)
    E   = clip(log r, >= log 1e-5) * (1-r)^2 (shared by both branches)
    raw = -(SS*t + w_c*(1-2t)) * E
    sum(raw) = -SS*sum(t*E) - sum_c w_c * colsum((1-2t)*E)_c

and with sgn = 2t-1 (exact in bf16):
    v = x + (t-1),  r = v * sgn = |v|   (abs as one 2x bf16 multiply)
    M = E * sgn = -(1-2t)*E

So each core only needs two per-class column sums, colsum(E) and colsum(M);
the host applies class weights and the -SS term (cs_tE = (cs_E + cs_M)/2).

Per-core layout: batch shard [65536, 20] viewed as [128 partitions x 10240],
where free index f has class (f mod 20).  Column sums run on the otherwise
idle TensorE with a ones[128,1] stationary operand, accumulating N=320
column blocks (320 % 20 == 0, <= one PSUM bank) into PSUM [1,320]
accumulators, so PSUM column j holds class (j mod 20).

Engine budget per core (10240 elem/partition, measured DMA floor ~21us):
  DMA   x + t loads, 10.49 MB HBM                    ~21us  <- roofline
  GPSIMD 1-input casts/affines: xb, tm, sgn (+clamp) ~hidden
  DVE   4 bf16 2x tensor_tensor passes               ~21us
  ACT   Ln + Square (both in `natural_log` set)      ~18us
  PE    64 matmuls N=320 into 2 PSUM accumulators    ~hidden
"""

from contextlib import ExitStack

import numpy as np

import concourse.bacc as bacc
import concourse.tile as tile
from concourse import mybir
from concourse.bass_utils import run_bass_kernel_spmd

B, C = 524288, 20
NCORES = 8
BS = B // NCORES            # 65536 batch rows per core
P = 128                     # SBUF partitions
FD = BS * C // P            # 10240 free elements per partition
CHUNK = 2560                # free elems per pipeline chunk (multiple of 20)
NCH = FD // CHUNK           # 4 chunks
NMM = 320                   # matmul free size (multiple of 20, <=512)
SS = 210.0
LOG_LO = float(np.log(np.float32(1e-5)))    # -11.512925
W = np.arange(1, C + 1, dtype=np.float64)   # class weights

F32 = mybir.dt.float32
BF16 = mybir.dt.bfloat16
I32 = mybir.dt.int32
Alu = mybir.AluOpType
Act = mybir.ActivationFunctionType

# host-side scale applied to the second accumulator to recover
# cs_ME = colsum((1-2t) * E) for each variant
M2_SCALE = {"gp_u": 2.0, "v2": -1.0, "v3": 2.0, "v4": -1.0}
DEFAULT_VARIANT = "v3"


def build_bass(
    variant: str = DEFAULT_VARIANT,
    reps: int = 1,
    loop_n: int = 0,
    stages: int = 4,
    clamp_on: str = "gp",
    cast_dma: bool = False,
    bufs: tuple = (4, 4, 3, 2),
    chunk: int = 1280,
    gp_cols: int = 0,
) -> bacc.Bacc:
    """Per-core SPMD program.

    variant "v2": bf16 pipeline, abs via sign-multiply, GPSIMD does the
    1-input casts; "gp_u": f32 r, GPSIMD does the 2-input add (slower,
    kept as a proven-correct fallback).
    `reps` statically unrolls the body; `loop_n` > 0 wraps it in a dynamic
    For_i loop (timing amplification only).  `stages` < 4 ablates stages
    for engine attribution.
    """
    nc = bacc.Bacc(None, debug=False)
    x = nc.dram_tensor("output", [BS, C], F32, kind="ExternalInput")
    t = nc.dram_tensor("target", [BS, C], I32, kind="ExternalInput")
    out = nc.dram_tensor("partials", [1, 2 * NMM], F32, kind="ExternalOutput")

    xv = x[:].rearrange("(p f) c -> p (f c)", p=P)   # [128, 10240] f32
    tv = t[:].rearrange("(p f) c -> p (f c)", p=P)   # [128, 10240] i32

    b_in, b_cast, b_mid, b_out = bufs

    with ExitStack() as ctx:
        tc = ctx.enter_context(tile.TileContext(nc))
        singles = ctx.enter_context(tc.tile_pool(name="singles", bufs=1))
        apool = ctx.enter_context(tc.tile_pool(name="a", bufs=b_in))
        tpool = ctx.enter_context(tc.tile_pool(name="t", bufs=b_in))
        upool = ctx.enter_context(tc.tile_pool(name="u", bufs=b_cast))
        bpool = ctx.enter_context(tc.tile_pool(name="b", bufs=b_mid))
        opool = ctx.enter_context(tc.tile_pool(name="o", bufs=b_out))
        psum = ctx.enter_context(tc.tile_pool(name="ps", bufs=2, space="PSUM"))

        ones = singles.tile([P, 1], BF16)
        nc.vector.memset(ones, 1.0)

        def chunk_v2(ci, nch, ps_e, ps_m):
            sl = slice(ci * chunk, (ci + 1) * chunk)
            if cast_dma:
                # SWDGE casts during the DMA itself; no engine pass needed.
                xb = apool.tile([P, chunk], BF16, tag="xb")
                tb = tpool.tile([P, chunk], BF16, tag="tb")
                nc.gpsimd.dma_start(out=xb, in_=xv[:, sl])
                nc.gpsimd.dma_start(out=tb, in_=tv[:, sl])
            else:
                xt = apool.tile([P, chunk], F32, tag="xt")
                ti = tpool.tile([P, chunk], I32, tag="ti")
                nc.sync.dma_start(out=xt, in_=xv[:, sl])
                nc.sync.dma_start(out=ti, in_=tv[:, sl])
            if stages < 1:
                return
            if not cast_dma:
                xb = upool.tile([P, chunk], BF16, tag="xb")
                nc.gpsimd.tensor_copy(out=xb, in_=xt)
            # tm = t - 1 in {-1, 0};  sgn = 2t - 1 in {-1, +1}
            tm = upool.tile([P, chunk], BF16, tag="tm")
            sg = upool.tile([P, chunk], BF16, tag="sg")
            tsrc = tb if cast_dma else ti
            nc.gpsimd.tensor_scalar(
                out=tm, in0=tsrc, scalar1=-1.0, scalar2=None,
                op0=Alu.add, op1=Alu.bypass,
            )
            nc.gpsimd.tensor_scalar(
                out=sg, in0=tsrc, scalar1=2.0, scalar2=-1.0,
                op0=Alu.mult, op1=Alu.add,
            )
            # v = x + (t-1);  r = v * sgn = |x + t - 1|
            v = upool.tile([P, chunk], BF16, tag="v")
            nc.vector.tensor_add(v, xb, tm)
            r = upool.tile([P, chunk], BF16, tag="r")
            nc.vector.tensor_mul(r, v, sg)
            if stages < 2:
                return
            l0 = bpool.tile([P, chunk], BF16, tag="l0")
            nc.scalar.activation(l0, r, Act.Ln)
            s = bpool.tile([P, chunk], BF16, tag="s")
            nc.scalar.activation(s, r, Act.Square, bias=1.0, scale=-1.0)
            if stages < 3:
                return
            # lc = max(l0, log 1e-5)  (clamps the r==0 -> -inf case)
            lc = bpool.tile([P, chunk], BF16, tag="lc")
            if clamp_on == "gp":
                nc.gpsimd.tensor_scalar(
                    out=lc, in0=l0, scalar1=LOG_LO, scalar2=None,
                    op0=Alu.max, op1=Alu.bypass,
                )
            else:
                nc.vector.tensor_scalar(
                    out=lc, in0=l0, scalar1=LOG_LO, scalar2=None,
                    op0=Alu.max, op1=Alu.bypass,
                )
            e = bpool.tile([P, chunk], BF16, tag="e")
            nc.vector.tensor_mul(e, lc, s)
            m2 = bpool.tile([P, chunk], BF16, tag="m2")
            nc.vector.tensor_mul(m2, e, sg)
            if stages < 4:
                return
            for j in range(chunk // NMM):
                js = slice(j * NMM, (j + 1) * NMM)
                first = ci == 0 and j == 0
                last = ci == nch - 1 and j == chunk // NMM - 1
                nc.tensor.matmul(
                    ps_e[0:1, :], ones, e[:, js], start=first, stop=last
                )
                nc.tensor.matmul(
                    ps_m[0:1, :], ones, m2[:, js], start=first, stop=last
                )

        def chunk_gp_u(ci, nch, ps_e, ps_m):
            sl = slice(ci * chunk, (ci + 1) * chunk)
            xt = apool.tile([P, chunk], F32, tag="xt")
            ti = tpool.tile([P, chunk], I32, tag="ti")
            nc.sync.dma_start(out=xt, in_=xv[:, sl])
            nc.sync.dma_start(out=ti, in_=tv[:, sl])
            if stages < 1:
                return
            u = upool.tile([P, chunk], F32, tag="u")
            nc.gpsimd.tensor_add(u, xt, ti)
            if stages < 2:
                return
            r = upool.tile([P, chunk], F32, tag="r")
            nc.scalar.activation(r, u, Act.Abs, bias=1.0, scale=-1.0)
            l0 = bpool.tile([P, chunk], BF16, tag="l0")
            nc.scalar.activation(l0, r, Act.Ln)
            s = bpool.tile([P, chunk], BF16, tag="s")
            nc.scalar.activation(s, r, Act.Square, bias=1.0, scale=-1.0)
            if stages < 3:
                return
            g = bpool.tile([P, chunk], BF16, tag="g")
            nc.vector.tensor_scalar(
                out=g, in0=ti, scalar1=-1.0, scalar2=0.5,
                op0=Alu.mult, op1=Alu.add,
            )
            e = bpool.tile([P, chunk], BF16, tag="e")
            nc.vector.scalar_tensor_tensor(
                out=e, in0=l0, scalar=LOG_LO, in1=s,
                op0=Alu.max, op1=Alu.mult,
            )
            m2 = bpool.tile([P, chunk], BF16, tag="m2")
            nc.vector.tensor_mul(m2, e, g)
            if stages < 4:
                return
            for j in range(chunk // NMM):
                js = slice(j * NMM, (j + 1) * NMM)
                first = ci == 0 and j == 0
                last = ci == nch - 1 and j == chunk // NMM - 1
                nc.tensor.matmul(
                    ps_e[0:1, :], ones, e[:, js], start=first, stop=last
                )
                nc.tensor.matmul(
                    ps_m[0:1, :], ones, m2[:, js], start=first, stop=last
                )

        def chunk_v3(ci, nch, ps_e, ps_m):
            sl = slice(ci * chunk, (ci + 1) * chunk)
            xt = apool.tile([P, chunk], F32, tag="xt")
            ti = tpool.tile([P, chunk], I32, tag="ti")
            nc.sync.dma_start(out=xt, in_=xv[:, sl])
            nc.sync.dma_start(out=ti, in_=tv[:, sl])
            if stages < 1:
                return
            # u = x + t, column-split between GPSIMD (its one fast op,
            # plain tensor_tensor add) and DVE (fused (x-0)+t stt).
            u = upool.tile([P, chunk], F32, tag="u")
            gc = min(gp_cols, chunk)
            if gc > 0:
                nc.gpsimd.tensor_add(u[:, 0:gc], xt[:, 0:gc], ti[:, 0:gc])
            if gc < chunk:
                nc.vector.scalar_tensor_tensor(
                    out=u[:, gc:chunk], in0=xt[:, gc:chunk], scalar=0.0,
                    in1=ti[:, gc:chunk], op0=Alu.add, op1=Alu.add,
                )
            # r = |1 - u|  (walrus has no DVE abs; ACT Abs is the only one)
            r = upool.tile([P, chunk], F32, tag="r")
            nc.scalar.activation(r, u, Act.Abs, bias=1.0, scale=-1.0)
            if stages < 2:
                return
            l0 = bpool.tile([P, chunk], BF16, tag="l0")
            nc.scalar.activation(l0, r, Act.Ln)
            s = bpool.tile([P, chunk], BF16, tag="s")
            nc.scalar.activation(s, r, Act.Square, bias=1.0, scale=-1.0)
            if stages < 3:
                return
            g = bpool.tile([P, chunk], BF16, tag="g")
            nc.vector.tensor_scalar(
                out=g, in0=ti, scalar1=-1.0, scalar2=0.5,
                op0=Alu.mult, op1=Alu.add,
            )
            # E = max(l0, log 1e-5) * s  (fused clamp+mult)
            e = bpool.tile([P, chunk], BF16, tag="e")
            nc.vector.scalar_tensor_tensor(
                out=e, in0=l0, scalar=LOG_LO, in1=s,
                op0=Alu.max, op1=Alu.mult,
            )
            m2 = bpool.tile([P, chunk], BF16, tag="m2")
            nc.vector.tensor_mul(m2, e, g)
            if stages < 4:
                return
            for j in range(chunk // NMM):
                js = slice(j * NMM, (j + 1) * NMM)
                first = ci == 0 and j == 0
                last = ci == nch - 1 and j == chunk // NMM - 1
                nc.tensor.matmul(
                    ps_e[0:1, :], ones, e[:, js], start=first, stop=last
                )
                nc.tensor.matmul(
                    ps_m[0:1, :], ones, m2[:, js], start=first, stop=last
                )

        def chunk_v4(ci, nch, ps_e, ps_m):
            """Cast-DMA loads (bf16 lands directly), all-bf16 DVE chain,
            abs column-split between ACT (Abs) and DVE (sign-multiply)."""
            sl = slice(ci * chunk, (ci + 1) * chunk)
            xb = apool.tile([P, chunk], BF16, tag="xb")
            tb = tpool.tile([P, chunk], BF16, tag="tb")
            nc.gpsimd.dma_start(out=xb, in_=xv[:, sl])   # f32 -> bf16
            nc.gpsimd.dma_start(out=tb, in_=tv[:, sl])   # i32 -> bf16
            if stages < 1:
                return
            # sgn = 2t - 1 in {-1, +1}
            sg = upool.tile([P, chunk], BF16, tag="sg")
            nc.vector.tensor_scalar(
                out=sg, in0=tb, scalar1=2.0, scalar2=-1.0,
                op0=Alu.mult, op1=Alu.add,
            )
            # v = (x - 1) + t  (fused; |v| = r)
            v = upool.tile([P, chunk], BF16, tag="v")
            nc.vector.scalar_tensor_tensor(
                out=v, in0=xb, scalar=1.0, in1=tb,
                op0=Alu.subtract, op1=Alu.add,
            )
            # r = |v|: ACT Abs for the first gp_cols columns, DVE v*sgn
            # for the rest — balances the two engines.
            r = upool.tile([P, chunk], BF16, tag="r")
            ac = min(gp_cols, chunk)
            if ac > 0:
                nc.scalar.activation(r[:, 0:ac], v[:, 0:ac], Act.Abs)
            if ac < chunk:
                nc.vector.tensor_mul(
                    r[:, ac:chunk], v[:, ac:chunk], sg[:, ac:chunk]
                )
            if stages < 2:
                return
            l0 = bpool.tile([P, chunk], BF16, tag="l0")
            nc.scalar.activation(l0, r, Act.Ln)
            s = bpool.tile([P, chunk], BF16, tag="s")
            nc.scalar.activation(s, r, Act.Square, bias=1.0, scale=-1.0)
            if stages < 3:
                return
            e = bpool.tile([P, chunk], BF16, tag="e")
            nc.vector.scalar_tensor_tensor(
                out=e, in0=l0, scalar=LOG_LO, in1=s,
                op0=Alu.max, op1=Alu.mult,
            )
            m2 = bpool.tile([P, chunk], BF16, tag="m2")
            nc.vector.tensor_mul(m2, e, sg)
            if stages < 4:
                return
            for j in range(chunk // NMM):
                js = slice(j * NMM, (j + 1) * NMM)
                first = ci == 0 and j == 0
                last = ci == nch - 1 and j == chunk // NMM - 1
                nc.tensor.matmul(
                    ps_e[0:1, :], ones, e[:, js], start=first, stop=last
                )
                nc.tensor.matmul(
                    ps_m[0:1, :], ones, m2[:, js], start=first, stop=last
                )

        chunk_fn = {
            "v2": chunk_v2, "gp_u": chunk_gp_u, "v3": chunk_v3, "v4": chunk_v4
        }[variant]
        nch = FD // chunk
        assert FD % chunk == 0 and chunk % NMM == 0 and chunk % 20 == 0

        def body():
            for rep in range(reps):
                ps_e = ps_m = None
                if stages >= 4:
                    ps_e = psum.tile([1, NMM], F32, tag="ps_e")
                    ps_m = psum.tile([1, NMM], F32, tag="ps_m")
                for ci in range(nch):
                    chunk_fn(ci, nch, ps_e, ps_m)
                res = opool.tile([1, 2 * NMM], F32, tag="res")
                if stages >= 4:
                    nc.vector.tensor_copy(res[0:1, 0:NMM], ps_e[0:1, :])
                    nc.vector.tensor_copy(res[0:1, NMM : 2 * NMM], ps_m[0:1, :])
                else:
                    nc.vector.memset(res, 0.0)
                nc.sync.dma_start(out=out[:], in_=res)

        if loop_n > 0:
            with tc.For_i(0, loop_n, 1):
                body()
        else:
            body()

    nc.finalize()
    return nc


_NC_CACHE: dict = {}


def _get_nc(**kw) -> bacc.Bacc:
    key = tuple(sorted(kw.items()))
    if key not in _NC_CACHE:
        _NC_CACHE[key] = build_bass(**kw)
    return _NC_CACHE[key]


def combine_partials(partials, m2_scale: float = M2_SCALE["v2"]) -> np.float32:
    """Host-side reduction of the per-core [1, 2*NMM] partial sums."""
    cs_e = np.zeros(C, dtype=np.float64)
    cs_m2 = np.zeros(C, dtype=np.float64)
    cols = np.arange(NMM) % C
    for p in partials:
        p = np.asarray(p, dtype=np.float64).reshape(2 * NMM)
        np.add.at(cs_e, cols, p[:NMM])
        np.add.at(cs_m2, cols, p[NMM:])
    cs_me = m2_scale * cs_m2            # colsum((1-2t) * E)
    cs_te = (cs_e - cs_me) / 2.0        # colsum(t * E)
    total = (-SS * cs_te - W * cs_me).sum()
    return np.float32(10.0 * total / (SS * B * C))


def kernel(output: np.ndarray, target: np.ndarray) -> np.ndarray:
    output = np.ascontiguousarray(np.asarray(output, dtype=np.float32))
    target = np.ascontiguousarray(np.asarray(target, dtype=np.int32))
    assert output.shape == (B, C) and target.shape == (B, C)

    nc = _get_nc()
    xs = output.reshape(NCORES, BS, C)
    ts = target.reshape(NCORES, BS, C)
    in_maps = [{"output": xs[i], "target": ts[i]} for i in range(NCORES)]
    res = run_bass_kernel_spmd(nc, in_maps, core_ids=list(range(NCORES)))
    return np.asarray(
        combine_partials(
            [res.results[i]["partials"] for i in range(NCORES)],
            m2_scale=M2_SCALE[DEFAULT_VARIANT],
        )
    )
